# revision 29
# baseline (speedup 1.0000x reference)
"""Low-rank attention Trainium2 kernel (8 NeuronCores, SPMD), bf16 edition.

Math (reference):
    tmp = relu(x @ W.T + b); U,V,Z,T = split(tmp, 4, axis=1)
    norm = sum(U @ colsum(V)) / n + eps ;  D = 1/norm
    out = concat[(U @ (V.T @ Z)) * D, T]

Sharding: rows of x across 8 cores. Per-core partials (V.T@[Z|1] [k,k+1]
which embeds colsum(V) in its last column, plus colsum(U)) are AllReduced
on-device; each core then computes its local U @ (VtZ * D).

Layout: everything except V/Z is computed TRANSPOSED ([feature, row]) so the
weight block is the stationary matmul operand and is reused across 4 moving
512-wide row chunks. V/Z need rows on partitions for the V^T@Z contraction,
so they alone use the natural layout (x-slice stationary). The kernel emits
outT [2k, n_loc]; the host transposes back. bf16 operands halve DMA and get
FWL weight loads; fp32 PSUM accumulation keeps rel-err ~5e-3.

All of x stays resident in SBUF (16 MB bf16), so the whole T-pass defers
until after the AllReduce launch with no HBM re-read, hiding the collective
behind ~55us of PE work.
"""
import sys

sys.path.insert(0, "/opt/trn_rl_repo")
import ml_dtypes
import numpy as np

NCORES = 8
N_ROWS, D_IN, K = 65536, 1024, 256
NLOC = N_ROWS // NCORES      # 8192 rows per core
P = 128
IG = 2048                    # i-group (rows handled per outer iteration)
NG = NLOC // IG              # 4 groups
IC = 512                     # i-chunk = one PSUM bank of fp32
NIC = IG // IC               # 4 chunks per group
NS = IG // P                 # 16 row-subtiles per group
EPS = 1e-6

_built = {}


def _build(d_rows, resident):
    import concourse.bacc as bacc
    import concourse.mybir as mybir
    import concourse.tile as tile

    dt = mybir.dt
    f32, bf16 = dt.float32, dt.bfloat16
    RELU = mybir.ActivationFunctionType.Relu
    DT = d_rows // P

    nc = bacc.Bacc("TRN2", target_bir_lowering=False, debug=False, num_devices=NCORES)
    xT = nc.dram_tensor("xT", [d_rows, NLOC], bf16, kind="ExternalInput")
    WT = nc.dram_tensor("WT", [d_rows, 4 * K], bf16, kind="ExternalInput")
    outT = nc.dram_tensor("outT", [2 * K, NLOC], bf16, kind="ExternalOutput")

    with tile.TileContext(nc) as tc:
        with (
            tc.tile_pool(name="wp", bufs=1) as wp,
            tc.tile_pool(name="xp", bufs=1) as xp,
            tc.tile_pool(name="up", bufs=1) as up,
            tc.tile_pool(name="vzp", bufs=5) as vzp,
            tc.tile_pool(name="op", bufs=6) as op,
            tc.tile_pool(name="acc", bufs=1) as accp,
            tc.tile_pool(name="ps", bufs=6, space="PSUM") as ps,
            tc.tile_pool(name="psv", bufs=2, space="PSUM") as psv,
            tc.tile_pool(name="dram", bufs=1, space="DRAM") as dram,
        ):
            wt = []
            for kd in range(DT):
                w = wp.tile([P, 4 * K], bf16, tag=f"w{kd}", name=f"w{kd}")
                nc.gpsimd.dma_start(out=w[:], in_=WT[kd * P:(kd + 1) * P, :])
                wt.append(w)


            ut = [up.tile([P, NLOC], bf16, tag=f"ut{h}", name=f"ut{h}") for h in range(2)]
            csu_cols = [
                accp.tile([P, NG * NIC], f32, tag=f"csuc{h}", name=f"csuc{h}")
                for h in range(2)
            ]
            # collective payload, assembled in-place: [vtz0|csv0, vtz1|csv1, csu0, csu1]
            binsb = accp.tile([P, 2 * (K + 1) + 2], f32, tag="binsb")
            binsb16 = accp.tile([P, 2 * (K + 1) + 2], bf16, tag="binsb16")
            vtz_acc = [binsb[:, h * (K + 1):(h + 1) * (K + 1)] for h in range(2)]

            # x tiles: resident path holds the whole shard in SBUF.
            def new_xtiles(g):
                xt = []
                for kd in range(DT):
                    t = xp.tile(
                        [P, IG], bf16,
                        tag=(f"x{g}_{kd}" if resident else f"x{kd}"),
                        bufs=(1 if resident else 2),
                        name=f"x{g}_{kd}",
                    )
                    nc.sync.dma_start(
                        out=t[:], in_=xT[kd * P:(kd + 1) * P, g * IG:(g + 1) * IG]
                    )
                    xt.append(t)
                return xt

            xg = [new_xtiles(g) for g in range(NG)] if resident else [None] * NG

            # transposed-layout projection for one 128-wide feature block jb:
            # psum[jb, ic] += wt[kd][:,jb].T @ xt[kd][:,ic]  (weight stationary,
            # reused across the NIC moving chunks)
            def tpass(g, jb, xt):
                pts = [ps.tile([P, IC], f32, tag="work", name=f"pt{i}") for i in range(NIC)]
                for kd in range(DT):
                    for ic in range(NIC):
                        nc.tensor.matmul(
                            pts[ic][:], wt[kd][:, jb * P:(jb + 1) * P],
                            xt[kd][:, ic * IC:(ic + 1) * IC],
                            start=(kd == 0), stop=(kd == DT - 1),
                            skip_group_check=True,
                        )
                for ic in range(NIC):
                    i0 = g * IG + ic * IC
                    if jb < 2:  # U features, keep transposed in SBUF + colsum(U)
                        nc.scalar.activation(
                            ut[jb][:, i0:i0 + IC], pts[ic][:], RELU,
                            accum_out=csu_cols[jb][:, g * NIC + ic:g * NIC + ic + 1],
                        )
                    else:       # T features, straight to output rows 256:512
                        ot = op.tile([P, IC], bf16, tag="ot", name="ot")
                        nc.vector.tensor_relu(ot[:], pts[ic][:])
                        nc.scalar.dma_start(
                            out=outT[K + (jb - 6) * P:K + (jb - 5) * P, i0:i0 + IC],
                            in_=ot[:],
                        )

            # ---- phase 1: projection + VtZ/colsum partials ----
            for g in range(NG):
                xt = xg[g] if resident else new_xtiles(g)
                for jb in ([0, 1] if resident else [0, 1, 6, 7]):
                    tpass(g, jb, xt)
                # natural-layout V|Z for this group's 16 row-subtiles, with a
                # ones column so V^T@[Z|1] also yields colsum(V) in column K.
                pvtz = [
                    psv.tile([P, K + 1], f32, tag="vtz", name=f"pvtz{h}")
                    for h in range(2)
                ]
                # VtZ matmuls run one subtile behind the VZ pass so each
                # relu has a full subtile of slack before the PE needs it
                pend = []
                def flush_vtz(last=False):
                    for idx, vzt in pend:
                        for h in range(2):
                            nc.tensor.matmul(
                                pvtz[h][:], vzt[:, h * P:(h + 1) * P],
                                vzt[:, K:2 * K + 1],
                                start=(idx == 0), stop=(last and idx == NS - 1),
                                skip_group_check=True,
                            )
                    pend.clear()
                for s in range(NS):
                    pvz = ps.tile([P, 2 * K], f32, tag="work", name="pvz")
                    for kd in range(DT):
                        nc.tensor.matmul(
                            pvz[:], xt[kd][:, s * P:(s + 1) * P], wt[kd][:, K:3 * K],
                            start=(kd == 0), stop=(kd == DT - 1),
                        )
                    vz = vzp.tile([P, 2 * K + 1], bf16, tag="vz", name="vz")
                    nc.vector.tensor_relu(vz[:, 0:2 * K], pvz[:])
                    nc.vector.memset(vz[:, 2 * K:2 * K + 1], 1.0)
                    if pend:
                        flush_vtz()
                    pend.append((s, vz))
                flush_vtz(last=True)
                for h in range(2):
                    if g == 0:
                        nc.vector.tensor_copy(vtz_acc[h], pvtz[h][:])
                    else:
                        nc.vector.tensor_add(vtz_acc[h], vtz_acc[h], pvtz[h][:])

            # ---- phase 2: AllReduce one contiguous [P, 2(K+1)+2] block ----
            for h in range(2):
                nc.vector.reduce_sum(
                    binsb[:, 2 * (K + 1) + h:2 * (K + 1) + h + 1],
                    csu_cols[h][:], axis=mybir.AxisListType.X,
                )
            nc.scalar.copy(binsb16[:], binsb[:])
            bin_ = dram.tile([P, 2 * (K + 1) + 2], bf16, name="bin")
            bout = dram.tile([P, 2 * (K + 1) + 2], bf16, name="bout")
            nc.sync.dma_start(out=bin_[:, :], in_=binsb16[:])
            nc.gpsimd.collective_compute(
                "AllReduce", mybir.AluOpType.add,
                replica_groups=[list(range(NCORES))],
                ins=[bin_.opt()], outs=[bout.opt()],
            )

            # ---- deferred T-pass part 1: keeps PE busy through the AllReduce
            # (incl. ~20us inter-core skew observed on the mesh) ----
            if resident:
                for g, jb in ((0, 6), (0, 7), (1, 6), (1, 7), (2, 6), (2, 7), (3, 6)):
                    tpass(g, jb, xg[g])

            # ---- phase 3: D = 1/(csU.csV/n + eps); vtzr = VtZ * D ----
            vtzf_all = accp.tile([P, 2 * (K + 1) + 2], bf16, tag="vtzf")
            nc.sync.dma_start(out=vtzf_all[:], in_=bout[:, :])
            pdot = ps.tile([1, 1], f32, tag="work", name="pdot")
            for h in range(2):
                nc.tensor.matmul(
                    pdot[:], vtzf_all[:, 2 * (K + 1) + h:2 * (K + 1) + h + 1],
                    vtzf_all[:, h * (K + 1) + K:h * (K + 1) + K + 1],
                    start=(h == 0), stop=(h == 1),
                )
            dsb = accp.tile([1, 1], f32, tag="dsb")
            nc.vector.tensor_scalar(
                out=dsb[:], in0=pdot[:], scalar1=1.0 / N_ROWS, scalar2=EPS,
                op0=mybir.AluOpType.mult, op1=mybir.AluOpType.add,
            )
            nc.vector.reciprocal(dsb[:], dsb[:])
            dbc = accp.tile([P, 1], f32, tag="dbc")
            nc.gpsimd.partition_broadcast(dbc[:], dsb[:])

            # ---- phase 4: resT = VtZ.T @ UT (unscaled — D is applied during
            # the psum eviction, so the matmuls only wait on the bout DMA) ----
            for q in range(2):
                for icg in range(NIC):
                    prs = [
                        ps.tile([P, IC], f32, tag="work", name=f"pr{j}")
                        for j in range(4)
                    ]
                    for h in range(2):
                        for j in range(4):
                            ic = icg * 4 + j
                            nc.tensor.matmul(
                                prs[j][:],
                                vtzf_all[:, h * (K + 1) + q * P:h * (K + 1) + (q + 1) * P],
                                ut[h][:, ic * IC:(ic + 1) * IC],
                                start=(h == 0), stop=(h == 1),
                                skip_group_check=True,
                            )
                    for j in range(4):
                        ic = icg * 4 + j
                        orow = op.tile([P, IC], bf16, tag="ot", name="orow")
                        # alternate engines: eviction (× D) + DMA issue both
                        # keep up with one res tile per 4 matmuls
                        if j % 2 == 0:
                            nc.scalar.mul(orow[:], prs[j][:], dbc[:])
                            nc.scalar.dma_start(
                                out=outT[q * P:(q + 1) * P, ic * IC:(ic + 1) * IC],
                                in_=orow[:],
                            )
                        else:
                            nc.vector.tensor_scalar_mul(orow[:], prs[j][:], dbc[:])
                            nc.sync.dma_start(
                                out=outT[q * P:(q + 1) * P, ic * IC:(ic + 1) * IC],
                                in_=orow[:],
                            )

            # ---- deferred T-pass part 2: overlaps the res output burst ----
            if resident:
                tpass(3, 7, xg[3])

    _strip_redundant_ldweights(nc, mybir)
    nc.compile()
    return nc


def _strip_redundant_ldweights(nc, mybir):
    """Drop InstLdweights that reload the exact weights AP already in the PE
    array (only matmuls in between, no semaphore waits/updates attached).
    Legalization emits one load per matmul unconditionally; a reload of the
    identical region is dead time (~107ns each, only half hidden by the
    previous matmul's drain)."""
    for blk in nc.main_func.blocks:
        new_insts = []
        last_sig = None
        for inst in blk.instructions:
            tn = type(inst).__name__
            if tn == "InstLdweights":
                si = inst.sync_info
                clean = si is None or (not si.on_wait and not si.on_update)
                sig = str(inst.ins[0])
                if clean and sig == last_sig:
                    continue
                last_sig = sig
            elif tn in ("InstMatmult", "InstMatmultMx"):
                # a matmul on other weights (e.g. self-loading fp32) clobbers
                # the array; so does transpose mode
                if getattr(inst, "is_transpose", False) or (
                    len(inst.ins) > 1 and str(inst.ins[1]) != last_sig
                ):
                    last_sig = None
            elif getattr(inst, "engine", None) == mybir.EngineType.PE:
                last_sig = None
            new_insts.append(inst)
        if len(new_insts) != len(blk.instructions):
            blk.instructions[:] = new_insts


def _get_nc(d_rows, resident):
    key = (d_rows, resident)
    if key not in _built:
        _built[key] = _build(d_rows, resident)
    return _built[key]


def _run(x, W, b, trace=False, trace_cores=None):
    from concourse.bass_utils import run_bass_kernel_spmd

    bf16 = ml_dtypes.bfloat16
    x = np.ascontiguousarray(x, dtype=np.float32)
    W = np.ascontiguousarray(W, dtype=np.float32)
    b = np.asarray(b, dtype=np.float32)
    if np.any(b):
        # pad contraction: a ones-row in x picks up b from an extra W row
        d_rows, resident = 1152, False
        WT_full = np.zeros((d_rows, 4 * K), bf16)
        WT_full[:D_IN] = W.T.astype(bf16)
        WT_full[D_IN] = b.astype(bf16)
    else:
        d_rows, resident = D_IN, True
        WT_full = np.ascontiguousarray(W.T).astype(bf16)
    nc = _get_nc(d_rows, resident)
    in_maps = []
    for c in range(NCORES):
        xs = x[c * NLOC:(c + 1) * NLOC]
        if resident:
            xTs = np.ascontiguousarray(xs.T).astype(bf16)
        else:
            xTs = np.zeros((d_rows, NLOC), bf16)
            xTs[:D_IN] = xs.T.astype(bf16)
            xTs[D_IN] = 1.0
        in_maps.append({"xT": xTs, "WT": WT_full})
    res = run_bass_kernel_spmd(
        nc, in_maps, list(range(NCORES)),
        trace=trace, **({"trace_cores": trace_cores} if trace_cores else {}),
    )
    full = np.concatenate(
        [np.ascontiguousarray(res.results[c]["outT"].T.astype(np.float32)) for c in range(NCORES)],
        axis=0,
    )
    return full, res


def kernel(x, W, b):
    full, _ = _run(x, W, b)
    return full
